# revision 19
# baseline (speedup 1.0000x reference)
"""Trainium2 Bass kernel for nn_Conv2d_85830626443584.

Math (from the reference):
  x: [16, 64, 128, 128] f32, W: [8, 9] f32
  s = silu(x)
  out[b, c*8+k, ho, wo] = sum_{dh,dw} W[k, 3*dh+dw] * s[b, c, ho+dh, wo+dw]
  out: [16, 512, 126, 126] f32

Strategy (per NeuronCore, batch-sharded 16/8 = 2 batches -> 128 channel-images):
  * Each channel-image is an independent [128, 128] tile, SBUF layout
    [partition=h, free=w].
  * The 3x3 conv is computed as 3 PSUM-accumulating matmuls per output map k:
    a banded stationary matrix Band[(h_in=128), (ho=126)] carries the 3
    vertical taps (dh); the horizontal taps (dw) come for free as rhs
    access-pattern column offsets.
  * Matmuls run in f32r (full PE rate, 1 col/cycle; bf16 operands fail the
    2e-2 rel-err gate). PSUM accumulates f32. Groups are processed in quads
    sharing each (k, dw) stationary so the ~224ns f32r weight reloads
    overlap consecutive matmuls.
  * The input is uploaded pre-transposed to [h, img, w] so group loads are
    2 KiB-contiguous per partition; the output is stored in the SBUF-native
    layout [g, ho, k, i, wo] as bf16 (8 KiB contiguous per partition, half
    the f32 store traffic, +-0.2% rounding) and the final [B, C*K, HO, WO]
    f32 arrangement is done on host.
  * PSUM drains (the only psum->SBUF path) are split between DVE and ACT;
    both engines run ~1 elem/cycle on f32 psum reads, so one engine alone
    would bottleneck the kernel.
"""

import numpy as np

B, C, H, WD = 16, 64, 128, 128
NK = 8            # n_convs
HO = WO = 126     # output spatial dims
NCORES = 8
B_LOC = B // NCORES              # 2 batches per core
NIMG_LOC = B_LOC * C             # 128 images per core
GRP = 4                          # images per group
NGRP = NIMG_LOC // GRP           # 32 groups
NDVE_K = 5                       # k-maps drained by DVE (rest by ACT)

_CACHE = {}


def _make_bands(W: np.ndarray) -> np.ndarray:
    """Banded stationary matrices, one [128, 126] per (k, dw).

    bands[h, k, dw, ho] = W[k, 3*dh + dw] where dh = h - ho in {0,1,2}.
    Returned flattened to [128, 8*3*126].
    """
    bands = np.zeros((H, NK, 3, HO), dtype=np.float32)
    ho = np.arange(HO)
    for dh in range(3):
        for dw in range(3):
            bands[ho + dh, :, dw, ho] = W[:, 3 * dh + dw][None, :]
    return bands.reshape(H, NK * 3 * HO)


def _build_module(native_silu: bool = True, ngrp: int = NGRP,
                  ndve_k: int = NDVE_K, gquad: int = 4):
    # native_silu=True: single ACT Silu instruction (hardware path). False:
    # Sigmoid + DVE mul, for CoreSim (which lacks a Silu implementation).
    # gquad: groups sharing one LDWEIGHTS per (k, dw) stationary — f32r
    # weight loads (~224ns) exceed one matmul's stream time (~210ns), so
    # back-to-back matmuls per stationary are needed to hide them.
    import concourse.mybir as mybir
    import concourse.tile as tile
    from concourse import bacc
    from contextlib import ExitStack

    f32 = mybir.dt.float32
    f32r = mybir.dt.float32r
    f16 = mybir.dt.float16
    bf16 = mybir.dt.bfloat16

    nc = bacc.Bacc("TRN2", target_bir_lowering=False, debug=False)

    # Input pre-transposed on host: [h, img, w] so each group load is one
    # 2 KiB-contiguous run per partition.
    x_d = nc.dram_tensor("x", [H, NIMG_LOC, WD], f32, kind="ExternalInput")
    bands_d = nc.dram_tensor("bands", [H, NK * 3 * HO], f32r, kind="ExternalInput")
    # Output in SBUF-native layout: [g, ho, k, i, wo] flattened to 2D.
    out_d = nc.dram_tensor(
        "out", [NGRP * HO, NK * GRP * WO], bf16, kind="ExternalOutput"
    )

    assert ngrp % gquad == 0

    with tile.TileContext(nc) as tc, ExitStack() as ctx:
        cpool = ctx.enter_context(tc.tile_pool(name="const", bufs=1))
        xpool = ctx.enter_context(tc.tile_pool(name="xin", bufs=2 * gquad))
        spool = ctx.enter_context(tc.tile_pool(name="silu", bufs=gquad + 2))
        opool = ctx.enter_context(tc.tile_pool(name="outs", bufs=gquad + 2))
        ppool = ctx.enter_context(tc.tile_pool(name="psum", bufs=8, space="PSUM"))

        band_t = cpool.tile([H, NK * 3 * HO], f32r)
        nc.sync.dma_start(band_t[:], bands_d.ap())
        band4 = band_t[:].rearrange("p (k d m) -> p k d m", k=NK, d=3)

        out2 = out_d.ap()

        for gq in range(ngrp // gquad):
            sts = []
            ots = []
            for j in range(gquad):
                g = gq * gquad + j
                i0 = g * GRP
                xt = xpool.tile([H, GRP * WD], f32)
                nc.sync.dma_start(
                    xt[:].rearrange("h (i w) -> h i w", i=GRP),
                    x_d.ap()[:, i0 : i0 + GRP, :],
                )
                st = spool.tile([H, GRP * WD], f32r, tag="st")
                if native_silu:
                    nc.scalar.activation(
                        st[:], xt[:], mybir.ActivationFunctionType.Silu
                    )
                else:
                    sg = spool.tile([H, GRP * WD], f32, tag="sg")
                    nc.scalar.activation(
                        sg[:], xt[:], mybir.ActivationFunctionType.Sigmoid
                    )
                    nc.vector.tensor_mul(st[:], xt[:], sg[:])
                sts.append(st[:].rearrange("h (i w) -> h i w", i=GRP))
                ot = opool.tile([HO, NK * GRP * WO], bf16)
                ots.append(ot)

            for k in range(NK):
                pss = []
                for j in range(gquad):
                    ps = ppool.tile([HO, GRP * WO], f32)
                    pss.append(ps[:].rearrange("p (i n) -> p i n", i=GRP))
                for dw in range(3):
                    for j in range(gquad):
                        nc.tensor.matmul(
                            pss[j],
                            band4[:, k, dw, :],
                            sts[j][:, :, dw : dw + WO],
                            start=(dw == 0),
                            stop=(dw == 2),
                        )
                for j in range(gquad):
                    ot4 = ots[j][:].rearrange(
                        "p (k i w) -> p k i w", k=NK, i=GRP
                    )
                    if k < ndve_k:
                        nc.vector.tensor_copy(ot4[:, k, :, :], pss[j])
                    else:
                        nc.scalar.activation(
                            ot4[:, k, :, :],
                            pss[j],
                            mybir.ActivationFunctionType.Copy,
                        )

            for j in range(gquad):
                g = gq * gquad + j
                nc.sync.dma_start(out2[g * HO : (g + 1) * HO, :], ots[j][:])

    nc.compile()
    return nc


NT = 8            # ho tiles of 16 (last covers 14)
TO = 16           # ho window per tile
JW = 18           # h' window per tile (TO + 2)
MB = 118          # shift-matmul m: even tile at j 0..53, odd tile at 64..117


def _make_shift(P: int, dw: int) -> np.ndarray:
    """Row-selector stationary [128, 118] for tile pair P, horizontal tap dw.

    Column dw*18+h' (even tile t=2P) selects input row h = 32P + h';
    column 64+dw*18+h' (odd tile) selects h = 32P + 16 + h'.
    Columns for other dw (and h >= 128 at the image bottom) are zero, so the
    3 dw matmuls accumulate disjoint partition blocks of one psum bank.
    """
    S = np.zeros((H, MB), dtype=np.float32)
    for par in range(2):
        base = 32 * P + 16 * par
        for hp in range(JW):
            if base + hp < H:
                S[base + hp, 64 * par + dw * JW + hp] = 1.0
    return S


def _make_conv_stationary(W: np.ndarray) -> np.ndarray:
    """Im2col conv stationary [128, 128]: rows (dw, h') doubled at offset 64,
    columns (k, o). Sc[dw*18+h', k*16+o] = W[k, 3*(h'-o)+dw] for h'-o in 0..2.
    Full 9-tap contraction per output in a single matmul."""
    Sc = np.zeros((H, 128), dtype=np.float32)
    for dw in range(3):
        for k in range(NK):
            for o in range(TO):
                for dh in range(3):
                    hp = o + dh
                    Sc[dw * JW + hp, k * TO + o] = W[k, 3 * dh + dw]
    Sc[64:64 + 54, :] = Sc[0:54, :]
    return Sc


def _build_module_b(native_silu: bool = True, ngrp: int = NGRP):
    """Design B: per ho-tile im2col. The silu'd image rows are replicated to
    the (dw, h') partition layout with 3 accumulating row-selector matmuls
    (one psum bank per tile pair), drained to SBUF as bf16, then ONE matmul
    per tile contracts all 9 taps at once (m = 8 k * 16 ho = 128). The two
    tiles of a pair run as concurrent row-tiled matmuls (rows 0:54 / 64:118).
    PE cycles/group: 12*504 shift + 4*504 compute-wall vs 24*504 for the
    banded design."""
    import concourse.mybir as mybir
    import concourse.tile as tile
    from concourse import bacc
    from contextlib import ExitStack

    f32 = mybir.dt.float32
    f32r = mybir.dt.float32r
    f16 = mybir.dt.float16
    bf16 = mybir.dt.bfloat16

    nc = bacc.Bacc("TRN2", target_bir_lowering=False, debug=False)

    x_d = nc.dram_tensor("x", [H, NIMG_LOC, WD], f32, kind="ExternalInput")
    # shift stationaries [4 pairs, 3 dw, 128, 118] flattened on free dim
    sh_d = nc.dram_tensor("sh", [H, 4 * 3 * MB], f32r, kind="ExternalInput")
    sc_d = nc.dram_tensor("sc", [H, 128], f32r, kind="ExternalInput")
    # out layout: [g, (k,o)=128, t, i, wo]
    out_d = nc.dram_tensor(
        "out", [NGRP * 128, NT * GRP * WO], bf16, kind="ExternalOutput"
    )

    with tile.TileContext(nc) as tc, ExitStack() as ctx:
        cpool = ctx.enter_context(tc.tile_pool(name="const", bufs=1))
        xpool = ctx.enter_context(tc.tile_pool(name="xin", bufs=3))
        spool = ctx.enter_context(tc.tile_pool(name="silu", bufs=3))
        mpool = ctx.enter_context(tc.tile_pool(name="tmps", bufs=3))
        opool = ctx.enter_context(tc.tile_pool(name="outs", bufs=3))
        tppool = ctx.enter_context(tc.tile_pool(name="ptmp", bufs=2, space="PSUM"))
        cppool = ctx.enter_context(tc.tile_pool(name="pcmp", bufs=2, space="PSUM"))

        sh_t = cpool.tile([H, 4 * 3 * MB], f32r)
        nc.sync.dma_start(sh_t[:], sh_d.ap())
        sh4 = sh_t[:].rearrange("p (q d m) -> p q d m", q=4, d=3)
        sc_t = cpool.tile([H, 128], f32r)
        nc.sync.dma_start(sc_t[:], sc_d.ap())

        out2 = out_d.ap()

        for g in range(ngrp):
            i0 = g * GRP
            xt = xpool.tile([H, GRP * WD], f32)
            nc.sync.dma_start(
                xt[:].rearrange("h (i w) -> h i w", i=GRP),
                x_d.ap()[:, i0 : i0 + GRP, :],
            )

            st = spool.tile([H, GRP * WD], f32r, tag="st")
            if native_silu:
                nc.scalar.activation(
                    st[:], xt[:], mybir.ActivationFunctionType.Silu
                )
            else:
                sg = spool.tile([H, GRP * WD], bf16, tag="sg")
                nc.scalar.activation(
                    sg[:], xt[:], mybir.ActivationFunctionType.Sigmoid
                )
                nc.vector.tensor_mul(st[:], xt[:], sg[:])
            st3 = st[:].rearrange("h (i w) -> h i w", i=GRP)

            ot = opool.tile([128, NT * GRP * WO], bf16)
            ot4 = ot[:].rearrange("p (t i w) -> p t i w", t=NT, i=GRP)

            for half in range(2):  # pairs (0,1) then (2,3)
                # one dual-bank psum tile holds both pairs' im2col rows
                tp = tppool.tile([128, 1024], f32)
                tp3 = tp[:].rearrange("p (q n) -> p q n", q=2)
                ts = mpool.tile([128, 2 * GRP * WO], f32r)
                ts3 = ts[:].rearrange("p (q i w) -> p q i w", q=2, i=GRP)
                for q in range(2):  # pair index within half
                    P = half * 2 + q
                    for dw in range(3):
                        nc.tensor.matmul(
                            tp3[0:MB, q, 0 : GRP * WO].rearrange(
                                "p (i n) -> p i n", i=GRP
                            ),
                            sh4[:, P, dw, :],
                            st3[:, :, dw : dw + WO],
                            start=(dw == 0),
                            stop=(dw == 2),
                            skip_group_check=True,
                        )
                    # drain this pair's bank immediately — it overlaps the
                    # other pair's shift matmuls, so the compute matmuls
                    # don't stall on a combined end-of-half drain. One
                    # engine per bank (parallel psum reads need distinct
                    # banks).
                    src_q = tp3[0:MB, q, 0 : GRP * WO].rearrange(
                        "p (i n) -> p i n", i=GRP
                    )
                    if q == 0:
                        nc.vector.tensor_copy(ts3[0:MB, q], src_q)
                    else:
                        nc.scalar.activation(
                            ts3[0:MB, q], src_q,
                            mybir.ActivationFunctionType.Copy,
                        )
                for q in range(2):
                    P = half * 2 + q
                    cp = cppool.tile([128, 1024], f32)
                    cp3 = cp[:].rearrange("p (b n) -> p b n", b=2)
                    for par in range(2):  # concurrent row-tiled pair
                        nc.tensor.matmul(
                            cp3[:, par, 0 : GRP * WO].rearrange(
                                "p (i n) -> p i n", i=GRP
                            ),
                            sc_t[64 * par : 64 * par + 54, :],
                            ts3[64 * par : 64 * par + 54, q, :, :],
                            start=True,
                            stop=True,
                            tile_position=(64 * par, 0),
                            skip_group_check=True,
                        )
                    # drain the pair's two output banks in one instruction
                    dst = ot4[:, 2 * P : 2 * P + 2, :, :]
                    src = cp3[:, :, 0 : GRP * WO].rearrange(
                        "p b (i n) -> p b i n", i=GRP
                    )
                    if P % 2 == 0:
                        nc.vector.tensor_copy(dst, src)
                    else:
                        nc.scalar.activation(
                            dst, src, mybir.ActivationFunctionType.Copy
                        )

            (nc.sync if g % 2 == 0 else nc.scalar).dma_start(
                out2[g * 128 : (g + 1) * 128, :], ot[:]
            )

    nc.compile()
    return nc


def _prep_inputs_b(x: np.ndarray, W: np.ndarray):
    xs = x.reshape(NCORES, B_LOC, C, H, WD)
    xh = np.ascontiguousarray(xs.transpose(0, 3, 1, 2, 4)).reshape(
        NCORES, H, NIMG_LOC, WD
    )
    sh = np.stack(
        [[_make_shift(P, dw) for dw in range(3)] for P in range(4)]
    )  # [4, 3, 128, 118]
    sh = np.ascontiguousarray(sh.transpose(2, 0, 1, 3)).reshape(H, 4 * 3 * MB)
    sc = _make_conv_stationary(W)
    return xh, np.ascontiguousarray(sh), sc


def _unpack_out_b(res_list):
    """[8 cores][g*128, t*i*wo] bf16 -> [16, 512, 126, 126] f32."""
    a = np.stack(res_list)  # [8, NGRP*128, NT*GRP*WO]
    a = a.reshape(NCORES, B_LOC, NGRP // B_LOC, NK, TO, NT, GRP, WO)
    # [core, b_loc, g2, k, o, t, i, wo] -> [core, b_loc, g2, i, k, t, o, wo]
    a = a.transpose(0, 1, 2, 6, 3, 5, 4, 7)
    a = a.reshape(B, C * NK, NT * TO, WO)[:, :, :HO, :]
    return np.ascontiguousarray(a).astype(np.float32)


def _get_module():
    if "nc" not in _CACHE:
        _CACHE["nc"] = _build_module()
    return _CACHE["nc"]


def _get_module_b():
    if "ncb" not in _CACHE:
        _CACHE["ncb"] = _build_module_b()
    return _CACHE["ncb"]


def _prep_inputs(x: np.ndarray, W: np.ndarray):
    """Host-side marshaling: shard f32 x, transpose to [h, img, w]."""
    # [16, 64, 128, 128] -> [8 cores, 2, 64, 128, 128] -> [8, h, (b c), w]
    xs = x.reshape(NCORES, B_LOC, C, H, WD)
    xh = np.ascontiguousarray(xs.transpose(0, 3, 1, 2, 4)).reshape(
        NCORES, H, NIMG_LOC, WD
    )
    bands = _make_bands(W)
    return xh, bands


def _unpack_out(res_list):
    """[8 cores][g*ho, k*i*wo] bf16 -> [16, 512, 126, 126] f32."""
    a = np.stack(res_list)  # [8, NGRP*HO, NK*GRP*WO]
    a = a.reshape(NCORES, B_LOC, NGRP // B_LOC, HO, NK, GRP, WO)
    # target channel = c*NK + k where c = g2*GRP + i  (g = b_loc*16 + g2)
    a = a.transpose(0, 1, 2, 5, 4, 3, 6)  # [core, b_loc, g2, i, k, ho, wo]
    a = a.reshape(B, C * NK, HO, WO)
    return np.asarray(a, dtype=np.float32)


def kernel(x: np.ndarray, W: np.ndarray) -> np.ndarray:
    from concourse.bass_utils import run_bass_kernel_spmd

    x = np.ascontiguousarray(np.asarray(x, dtype=np.float32))
    W = np.asarray(W, dtype=np.float32)
    assert x.shape == (B, C, H, WD), x.shape
    assert W.shape == (NK, 9), W.shape

    xh, sh, sc = _prep_inputs_b(x, W)
    nc = _get_module_b()

    in_maps = [{"x": xh[i], "sh": sh, "sc": sc} for i in range(NCORES)]
    res = run_bass_kernel_spmd(nc, in_maps, core_ids=list(range(NCORES)))
    return _unpack_out_b([res.results[i]["out"] for i in range(NCORES)])
